# revision 12
# baseline (speedup 1.0000x reference)
"""Hashed-weight MLP (1024-4096-4096-32000, batch 2048) on 8 TRN2 NeuronCores.

Problem: h = relu(x @ W0); h = relu(h @ W1); out = h @ W2, where each
W_l[i, j] = hw_l[(a_l*i + b_l*j + c_l) % N_l] is a virtual (ROBE-Z hashed)
weight gathered from a small parameter vector.

Approach (column-parallel tensor parallelism on all three layers):
  * Since N_l is a power of two and b_l is odd, b_l is invertible mod N_l.
    Through the host-computed permuted table hw_bb[t] = hw[(b*t) % N], the
    virtual weight becomes ROW-CONTIGUOUS:
        W[i, j0+dj] = hw_bb[u_i + j0 + dj],   u_i = b^-1 (a*i + c) % N
    and row starts u_i form an arithmetic progression mod N with stride
    q = b^-1 a. A whole [in_dim x width] weight shard therefore materializes
    with a handful of 3-level strided DMAs (signed-residue ladder over q) -
    NO per-element gathers anywhere.
  * Each core owns a 1/8 column shard of every layer. Per-core shard offsets
    are absorbed into the host-side slice of hw_bb (keeping the device
    program SPMD-uniform). Activations stay transposed [features, batch].
  * GEMMs are bf16 with fp32 PSUM accumulation.
  * v4 scheduling (vs v1): ALL weight staging (W0, W1 halves, all four W2
    j-groups) is issued up front; the scalar ring carries staging + slab
    loads + output stores, the sync ring carries activations; ReLUs and
    PSUM->SBUF copies run on the VECTOR engine so PSUM drain never queues
    behind DMA dispatches; the output is bf16 (host upcasts).
"""
import sys
if "/opt/trn_rl_repo" not in sys.path:
    sys.path.insert(0, "/opt/trn_rl_repo")

import numpy as np
import ml_dtypes

import concourse.bass as bass
import concourse.bacc as bacc
import concourse.tile as tile
import concourse.mybir as mybir
from concourse.bass_utils import run_bass_kernel_spmd

N_CORES = 8
P = 128
NB = 512                      # moving free dim (batch tile)
BATCH = 2048
BT = BATCH // NB              # 4

LENS = [1024, 4096, 4096, 32000]
HASH_A = [9973, 10007, 10039]
HASH_B = [31013, 31019, 31039]
HASH_C = [557, 563, 569]
SIZES = [1048576, 1048576, 4194304]

JW = [512, 512, 4000]         # true per-core output shard width
WMAT = [512, 512, 4096]       # materialized width (L2 padded to 32 j-tiles)

BF = mybir.dt.bfloat16
F32 = mybir.dt.float32


def _plan_layer(l):
    N = SIZES[l]; a, b, ch = HASH_A[l], HASH_B[l], HASH_C[l]
    binv = pow(b, -1, N)
    q = (binv * a) % N
    u0 = (binv * ch) % N
    in_dim = LENS[l]; w = WMAT[l]
    best = None
    for k in range(1, min(in_dim, 600) + 1):
        r = (q * k) % N
        if r > N // 2:
            r -= N
        C1 = -(-in_dim // k)
        extra = q * (k - 1) + abs(r) * (C1 - 1)
        if best is None or extra < best[0]:
            best = (extra, k, C1, r)
    _, k, C1, r = best
    shift = max(0, -r * (C1 - 1))
    m_ext = shift + q * (k - 1) + max(r, 0) * (C1 - 1) + w + 64
    if l == 1:
        # L1 is materialized in two row-halves with an extra mod-N base
        # offset on the second half; cover it in the slice.
        m_ext += N
    return dict(N=N, a=a, b=b, ch=ch, q=q, u0=u0, k=k, C1=C1, r=r,
                shift=shift, m_ext=m_ext, rows=k * C1, in_dim=in_dim, w=w)


PLANS = [_plan_layer(l) for l in range(3)]
RG = [list(range(N_CORES))]


def build_nc():
    nc = bacc.Bacc("TRN2", target_bir_lowering=False, debug=False,
                   num_devices=N_CORES)

    xT_d = nc.dram_tensor("xT", [LENS[0], BATCH], BF, kind="ExternalInput").ap()
    hb = [nc.dram_tensor(f"hb{l}", [PLANS[l]["m_ext"]], BF,
                         kind="ExternalInput").ap() for l in range(3)]
    w_mat = [nc.dram_tensor(f"w{l}", [PLANS[l]["rows"], PLANS[l]["w"]], BF).ap()
             for l in range(1)]
    # W1 in two row-half tensors, driven from both HWDGE rings in parallel.
    HROWS = 43 * 48           # 2064 >= 2048 rows per half (k=43, C1_half=48)
    w1_h = [nc.dram_tensor(f"w1h{h}", [HROWS, 512], BF).ap() for h in range(2)]
    # L2 weight shard split into 8 j-group tensors (per-slab dep tracking)
    w2_jg = [nc.dram_tensor(f"w2jg{g}", [PLANS[2]["rows"], 512], BF).ap()
             for g in range(8)]
    # per-batch-tile activation chunks: local contribution + allgathered full
    h1c = [nc.dram_tensor(f"h1c{p}", [512, 2 * NB], BF).ap() for p in range(2)]
    h1f = [nc.dram_tensor(f"h1f{p}", [4096, 2 * NB], BF, addr_space="Shared").ap()
           for p in range(2)]
    h2c = [nc.dram_tensor(f"h2c{b}", [512, NB], BF).ap() for b in range(BT)]
    h2f = [nc.dram_tensor(f"h2f{b}", [4096, NB], BF, addr_space="Shared").ap()
           for b in range(BT)]
    out_d = nc.dram_tensor("outT", [4096, BATCH], BF, kind="ExternalOutput").ap()

    def matz_small(l, eng):
        """Materialize W0. dim0 = a <=16-count block of the i1 axis: walrus
        splits a DMA across SDMA engine slots by the outermost dim only when
        its count is <= 16, so this shape fans out 16-wide."""
        pl = PLANS[l]
        q, k, C1, r, w = pl["q"], pl["k"], pl["C1"], pl["r"], pl["w"]
        off = pl["shift"]
        for k0 in range(0, k, 16):
            kc = min(16, k - k0)
            src = bass.AP(hb[l].tensor, off + q * k0,
                          [[q, kc], [r, C1], [1, w]])
            dst = bass.AP(w_mat[l].tensor, w * k0,
                          [[w, kc], [k * w, C1], [1, w]])
            eng.dma_start(out=dst, in_=src)

    def matz1_half(h, eng):
        """Materialize W1 rows [2048h, 2048h+2064) from the periodic slice.
        Row i = 2048h + i0 + 43*i1; base offset (q*2048h) mod N."""
        pl = PLANS[1]
        q, k, r, w, N = pl["q"], pl["k"], pl["r"], pl["w"], pl["N"]
        C1h = 48
        off = pl["shift"] + (q * 2048 * h) % N
        for k0 in range(0, k, 16):
            kc = min(16, k - k0)
            src = bass.AP(hb[1].tensor, off + q * k0,
                          [[q, kc], [r, C1h], [1, w]])
            dst = bass.AP(w1_h[h].tensor, w * k0,
                          [[w, kc], [k * w, C1h], [1, w]])
            eng.dma_start(out=dst, in_=src)

    def matz2(jgs, eng):
        """Materialize L2 j-group slabs (1KB inner runs spread 16-wide)."""
        pl = PLANS[2]
        q, k, C1, r = pl["q"], pl["k"], pl["C1"], pl["r"]
        w = 512
        nchunk = 2
        step = -(-C1 // nchunk)
        for g in jgs:
            for ci in range(nchunk):
                c1a = ci * step
                c1b = min(C1, c1a + step)
                cnt = c1b - c1a
                src = bass.AP(hb[2].tensor,
                              pl["shift"] + g * w + r * c1a,
                              [[q, k], [r, cnt], [1, w]])
                dst = bass.AP(w2_jg[g].tensor, k * w * c1a,
                              [[w, k], [k * w, cnt], [1, w]])
                eng.dma_start(out=dst, in_=src)

    with tile.TileContext(nc) as tc, \
         tc.tile_pool(name="ps", bufs=8, space="PSUM") as psp, \
         tc.tile_pool(name="slab", bufs=2) as slabp, \
         nc.allow_non_contiguous_dma(reason="hash ladder materialization"):

        # ---- all weight staging issued up front.
        matz_small(0, nc.scalar)

        with tc.tile_pool(name="l1w", bufs=1) as l1wp, \
             tc.tile_pool(name="l1r", bufs=4) as l1rp, \
             tc.tile_pool(name="l1o", bufs=8) as l1op:
            with tc.tile_pool(name="l0", bufs=1) as l0p:
                xsb = [l0p.tile([P, BATCH], BF, name=f"xsb{t}")
                       for t in range(8)]
                w0sb = [l0p.tile([P, 512], BF, name=f"w0sb{t}")
                        for t in range(8)]
                h1sb = [l0p.tile([P, BATCH], BF, name=f"h1sb{j}")
                        for j in range(4)]
                for t in range(8):
                    nc.sync.dma_start(out=xsb[t][:],
                                      in_=xT_d[t * P:(t + 1) * P, :])
                    nc.sync.dma_start(out=w0sb[t][:],
                                      in_=w_mat[0][t * P:(t + 1) * P, :])

                matz1_half(0, nc.gpsimd)
                matz1_half(1, nc.scalar)

                matz2(list(range(8)), nc.scalar)
                slabs = {}

                # ---- Layer 0
                for b in range(BT):
                    for j in range(4):
                        ps = psp.tile([P, NB], F32, tag="ps",
                                      name=f"ps0_{b}_{j}")
                        for kt in range(8):
                            nc.tensor.matmul(
                                out=ps[:],
                                lhsT=w0sb[kt][:, j * P:(j + 1) * P],
                                rhs=xsb[kt][:, b * NB:(b + 1) * NB],
                                start=(kt == 0), stop=(kt == 7))
                        nc.vector.tensor_scalar_max(
                            out=h1sb[j][:, b * NB:(b + 1) * NB],
                            in0=ps[:], scalar1=0.0)
                    if b % 2 == 1:
                        pr = b // 2
                        for j in range(4):
                            nc.sync.dma_start(
                                out=h1c[pr][j * P:(j + 1) * P, :],
                                in_=h1sb[j][:, pr * 2 * NB:(pr + 1) * 2 * NB])
                        nc.gpsimd.collective_compute(
                            "AllGather", mybir.AluOpType.bypass,
                            replica_groups=RG,
                            ins=[h1c[pr].opt()], outs=[h1f[pr].opt()])

            # ---- Layer 1 (l0 pools closed)
            w1sb = [l1wp.tile([P, 512], BF, name=f"w1sb{t}")
                    for t in range(32)]
            for t in range(32):
                h, lk = (0, t) if t < 16 else (1, t - 16)
                nc.sync.dma_start(out=w1sb[t][:],
                                  in_=w1_h[h][lk * P:(lk + 1) * P, :])
            for pr in range(2):
                pss = [psp.tile([P, NB], F32, tag="ps",
                                name=f"ps1_{pr}_{g}") for g in range(8)]
                for kt in range(32):
                    rhs = l1rp.tile([P, 2 * NB], BF, tag="l1rhs",
                                    name=f"l1r_{pr}_{kt}")
                    nc.sync.dma_start(out=rhs[:],
                                      in_=h1f[pr][kt * P:(kt + 1) * P, :])
                    for b2 in range(2):
                        for j in range(4):
                            nc.tensor.matmul(
                                out=pss[b2 * 4 + j][:],
                                lhsT=w1sb[kt][:, j * P:(j + 1) * P],
                                rhs=rhs[:, b2 * NB:(b2 + 1) * NB],
                                start=(kt == 0), stop=(kt == 31))
                for b2 in range(2):
                    b = pr * 2 + b2
                    for j in range(4):
                        h2o = l1op.tile([P, NB], BF, tag="h2o",
                                        name=f"h2o_{b}_{j}")
                        nc.vector.tensor_scalar_max(
                            out=h2o[:], in0=pss[b2 * 4 + j][:], scalar1=0.0)
                        nc.sync.dma_start(out=h2c[b][j * P:(j + 1) * P, :],
                                          in_=h2o[:])
                    nc.gpsimd.collective_compute(
                        "AllGather", mybir.AluOpType.bypass,
                        replica_groups=RG,
                        ins=[h2c[b].opt()], outs=[h2f[b].opt()])

        # ---- Layer 2 (W2 slabbed by 512-wide j-group, h2 SBUF-resident)
        for g in range(2):
            slabs[g] = [slabp.tile([P, 512], BF, tag=f"w2slab{t}",
                                   name=f"w2s_{g}_{t}") for t in range(32)]
            for t in range(32):
                nc.sync.dma_start(out=slabs[g][t][:],
                                  in_=w2_jg[g][t * P:(t + 1) * P, :])
        with tc.tile_pool(name="h2res", bufs=1) as h2p, \
             tc.tile_pool(name="l2o", bufs=8) as l2op:
            h2res = [h2p.tile([P, BATCH], BF, name=f"h2r{kt}")
                     for kt in range(32)]
            for kt in range(32):
                for b in range(BT):
                    nc.sync.dma_start(
                        out=h2res[kt][:, b * NB:(b + 1) * NB],
                        in_=h2f[b][kt * P:(kt + 1) * P, :])
            for jg in range(8):
                slab = slabs[jg]
                for b in range(BT):
                    pss = [psp.tile([P, NB], F32, tag="ps",
                                    name=f"ps2_{jg}_{b}_{j}")
                           for j in range(4)]
                    for kt in range(32):
                        for j in range(4):
                            nc.tensor.matmul(
                                out=pss[j][:],
                                lhsT=slab[kt][:, j * P:(j + 1) * P],
                                rhs=h2res[kt][:, b * NB:(b + 1) * NB],
                                start=(kt == 0), stop=(kt == 31))
                    for j in range(4):
                        osb = l2op.tile([P, NB], BF, tag="l2out",
                                        name=f"l2o_{jg}_{b}_{j}")
                        nc.vector.tensor_copy(out=osb[:], in_=pss[j][:])
                        nc.sync.dma_start(
                            out=out_d[(jg * 4 + j) * P:(jg * 4 + j + 1) * P,
                                      b * NB:(b + 1) * NB],
                            in_=osb[:])
                # load slab jg+2 now: its slot (slab jg's) was just freed
                if jg + 2 < 8:
                    g = jg + 2
                    slabs[g] = [slabp.tile([P, 512], BF, tag=f"w2slab{t}",
                                           name=f"w2s_{g}_{t}")
                                for t in range(32)]
                    for t in range(32):
                        nc.sync.dma_start(
                            out=slabs[g][t][:],
                            in_=w2_jg[g][t * P:(t + 1) * P, :])

    nc.compile()
    return nc


_NC_CACHE = None


def _get_nc():
    global _NC_CACHE
    if _NC_CACHE is None:
        _NC_CACHE = build_nc()
    return _NC_CACHE


def _prep_inputs(x, hw0, hw1, hw2):
    """Host prep: transpose x, build per-core periodic permuted-table slices."""
    x = np.asarray(x, np.float32)
    hws = [np.asarray(hw0, np.float32), np.asarray(hw1, np.float32),
           np.asarray(hw2, np.float32)]
    xT = np.ascontiguousarray(x.T).astype(ml_dtypes.bfloat16)

    per_core_hb = [[None] * 3 for _ in range(N_CORES)]
    for l in range(3):
        pl = PLANS[l]
        N, b = pl["N"], pl["b"]
        m_ext = pl["m_ext"]
        jw = JW[l]
        t0 = pl["u0"] - pl["shift"]          # core-0 slice start (in t-space)
        span = m_ext + (N_CORES - 1) * jw
        t = t0 + np.arange(span, dtype=np.int64)
        shared = hws[l][(b * t) % N].astype(ml_dtypes.bfloat16)
        for c in range(N_CORES):
            per_core_hb[c][l] = shared[c * jw: c * jw + m_ext]
    in_maps = []
    for c in range(N_CORES):
        in_maps.append({
            "xT": xT,
            "hb0": per_core_hb[c][0],
            "hb1": per_core_hb[c][1],
            "hb2": per_core_hb[c][2],
        })
    return in_maps


def kernel(x, hw0, hw1, hw2, trace=False):
    nc = _get_nc()
    in_maps = _prep_inputs(x, hw0, hw1, hw2)
    res = run_bass_kernel_spmd(nc, in_maps, list(range(N_CORES)), trace=trace)
    outs = [res.results[c]["outT"][:JW[2], :] for c in range(N_CORES)]
    full = np.concatenate(outs, axis=0)         # [32000, 2048] bf16
    out = np.ascontiguousarray(full.T).astype(np.float32)
    kernel.last_results = res
    return out


# revision 13
# speedup vs baseline: 1.1479x; 1.1479x over previous
"""Hashed-weight MLP (1024-4096-4096-32000, batch 2048) on 8 TRN2 NeuronCores.

Problem: h = relu(x @ W0); h = relu(h @ W1); out = h @ W2, where each
W_l[i, j] = hw_l[(a_l*i + b_l*j + c_l) % N_l] is a virtual (ROBE-Z hashed)
weight gathered from a small parameter vector.

Approach (column-parallel tensor parallelism on all three layers):
  * Since N_l is a power of two and b_l is odd, b_l is invertible mod N_l.
    Through the host-computed permuted table hw_bb[t] = hw[(b*t) % N], the
    virtual weight becomes ROW-CONTIGUOUS:
        W[i, j0+dj] = hw_bb[u_i + j0 + dj],   u_i = b^-1 (a*i + c) % N
    and row starts u_i form an arithmetic progression mod N with stride
    q = b^-1 a. A whole [in_dim x width] weight shard therefore materializes
    with a handful of 3-level strided DMAs (signed-residue ladder over q) -
    NO per-element gathers anywhere.
  * Each core owns a 1/8 column shard of every layer. Per-core shard offsets
    are absorbed into the host-side slice of hw_bb (keeping the device
    program SPMD-uniform). Activations stay transposed [features, batch].
  * GEMMs are bf16 with fp32 PSUM accumulation.
  * v4 scheduling (vs v1): ALL weight staging (W0, W1 halves, all four W2
    j-groups) is issued up front; the scalar ring carries staging + slab
    loads + output stores, the sync ring carries activations; ReLUs and
    PSUM->SBUF copies run on the VECTOR engine so PSUM drain never queues
    behind DMA dispatches; the output is bf16 (host upcasts).
"""
import sys
if "/opt/trn_rl_repo" not in sys.path:
    sys.path.insert(0, "/opt/trn_rl_repo")

import numpy as np
import ml_dtypes

import concourse.bass as bass
import concourse.bacc as bacc
import concourse.tile as tile
import concourse.mybir as mybir
from concourse.bass_utils import run_bass_kernel_spmd

N_CORES = 8
P = 128
NB = 512                      # moving free dim (batch tile)
BATCH = 2048
BT = BATCH // NB              # 4

LENS = [1024, 4096, 4096, 32000]
HASH_A = [9973, 10007, 10039]
HASH_B = [31013, 31019, 31039]
HASH_C = [557, 563, 569]
SIZES = [1048576, 1048576, 4194304]

JW = [512, 512, 4000]         # true per-core output shard width
WMAT = [512, 512, 4096]       # materialized width (L2 padded to 32 j-tiles)

BF = mybir.dt.bfloat16
F32 = mybir.dt.float32


def _plan_layer(l):
    N = SIZES[l]; a, b, ch = HASH_A[l], HASH_B[l], HASH_C[l]
    binv = pow(b, -1, N)
    q = (binv * a) % N
    u0 = (binv * ch) % N
    in_dim = LENS[l]; w = WMAT[l]
    best = None
    for k in range(1, min(in_dim, 600) + 1):
        r = (q * k) % N
        if r > N // 2:
            r -= N
        C1 = -(-in_dim // k)
        extra = q * (k - 1) + abs(r) * (C1 - 1)
        if best is None or extra < best[0]:
            best = (extra, k, C1, r)
    _, k, C1, r = best
    shift = max(0, -r * (C1 - 1))
    m_ext = shift + q * (k - 1) + max(r, 0) * (C1 - 1) + w + 64
    if l == 1:
        # L1 is materialized in two row-halves with an extra mod-N base
        # offset on the second half; cover it in the slice.
        m_ext += N
    return dict(N=N, a=a, b=b, ch=ch, q=q, u0=u0, k=k, C1=C1, r=r,
                shift=shift, m_ext=m_ext, rows=k * C1, in_dim=in_dim, w=w)


PLANS = [_plan_layer(l) for l in range(3)]
RG = [list(range(N_CORES))]


def build_nc():
    nc = bacc.Bacc("TRN2", target_bir_lowering=False, debug=False,
                   num_devices=N_CORES)

    xT_d = nc.dram_tensor("xT", [LENS[0], BATCH], BF, kind="ExternalInput").ap()
    hb = [nc.dram_tensor(f"hb{l}", [PLANS[l]["m_ext"]], BF,
                         kind="ExternalInput").ap() for l in range(3)]
    w_mat = [nc.dram_tensor(f"w{l}", [PLANS[l]["rows"], PLANS[l]["w"]], BF).ap()
             for l in range(1)]
    # W1 in two row-half tensors, driven from both HWDGE rings in parallel.
    HROWS = 43 * 48           # 2064 >= 2048 rows per half (k=43, C1_half=48)
    w1_h = [nc.dram_tensor(f"w1h{h}", [HROWS, 512], BF).ap() for h in range(2)]
    # L2 weight shard split into 8 j-group tensors (per-slab dep tracking)
    w2_jg = [nc.dram_tensor(f"w2jg{g}", [PLANS[2]["rows"], 512], BF).ap()
             for g in range(8)]
    # per-batch-tile activation chunks: local contribution + allgathered full
    h1c = [nc.dram_tensor(f"h1c{p}", [512, 2 * NB], BF).ap() for p in range(2)]
    h1f = [nc.dram_tensor(f"h1f{p}", [4096, 2 * NB], BF, addr_space="Shared").ap()
           for p in range(2)]
    h2c = [nc.dram_tensor(f"h2c{b}", [512, NB], BF).ap() for b in range(BT)]
    h2f = [nc.dram_tensor(f"h2f{b}", [4096, NB], BF, addr_space="Shared").ap()
           for b in range(BT)]
    out_d = nc.dram_tensor("outT", [4096, BATCH], BF, kind="ExternalOutput").ap()

    def matz_small(l, eng):
        """Materialize W0. dim0 = a <=16-count block of the i1 axis: walrus
        splits a DMA across SDMA engine slots by the outermost dim only when
        its count is <= 16, so this shape fans out 16-wide."""
        pl = PLANS[l]
        q, k, C1, r, w = pl["q"], pl["k"], pl["C1"], pl["r"], pl["w"]
        off = pl["shift"]
        for k0 in range(0, k, 16):
            kc = min(16, k - k0)
            src = bass.AP(hb[l].tensor, off + q * k0,
                          [[q, kc], [r, C1], [1, w]])
            dst = bass.AP(w_mat[l].tensor, w * k0,
                          [[w, kc], [k * w, C1], [1, w]])
            eng.dma_start(out=dst, in_=src)

    def matz1_half(h, eng):
        """Materialize W1 rows [2048h, 2048h+2064) from the periodic slice.
        Row i = 2048h + i0 + 43*i1; base offset (q*2048h) mod N."""
        pl = PLANS[1]
        q, k, r, w, N = pl["q"], pl["k"], pl["r"], pl["w"], pl["N"]
        C1h = 48
        off = pl["shift"] + (q * 2048 * h) % N
        for k0 in range(0, k, 16):
            kc = min(16, k - k0)
            src = bass.AP(hb[1].tensor, off + q * k0,
                          [[q, kc], [r, C1h], [1, w]])
            dst = bass.AP(w1_h[h].tensor, w * k0,
                          [[w, kc], [k * w, C1h], [1, w]])
            eng.dma_start(out=dst, in_=src)

    def matz2(jgs, eng):
        """Materialize L2 j-group slabs (1KB inner runs spread 16-wide)."""
        pl = PLANS[2]
        q, k, C1, r = pl["q"], pl["k"], pl["C1"], pl["r"]
        w = 512
        nchunk = 2
        step = -(-C1 // nchunk)
        for g in jgs:
            for ci in range(nchunk):
                c1a = ci * step
                c1b = min(C1, c1a + step)
                cnt = c1b - c1a
                src = bass.AP(hb[2].tensor,
                              pl["shift"] + g * w + r * c1a,
                              [[q, k], [r, cnt], [1, w]])
                dst = bass.AP(w2_jg[g].tensor, k * w * c1a,
                              [[w, k], [k * w, cnt], [1, w]])
                eng.dma_start(out=dst, in_=src)

    with tile.TileContext(nc) as tc, \
         tc.tile_pool(name="ps", bufs=8, space="PSUM") as psp, \
         tc.tile_pool(name="slab", bufs=2) as slabp, \
         nc.allow_non_contiguous_dma(reason="hash ladder materialization"):

        # ---- all weight staging issued up front.
        matz_small(0, nc.scalar)

        with tc.tile_pool(name="l1w", bufs=1) as l1wp, \
             tc.tile_pool(name="l1r", bufs=4) as l1rp, \
             tc.tile_pool(name="l1o", bufs=8) as l1op:
            with tc.tile_pool(name="l0", bufs=1) as l0p:
                xsb = [l0p.tile([P, BATCH], BF, name=f"xsb{t}")
                       for t in range(8)]
                w0sb = [l0p.tile([P, 512], BF, name=f"w0sb{t}")
                        for t in range(8)]
                h1sb = [l0p.tile([P, BATCH], BF, name=f"h1sb{j}")
                        for j in range(4)]
                for t in range(8):
                    nc.sync.dma_start(out=xsb[t][:],
                                      in_=xT_d[t * P:(t + 1) * P, :])
                    nc.sync.dma_start(out=w0sb[t][:],
                                      in_=w_mat[0][t * P:(t + 1) * P, :])

                matz1_half(0, nc.scalar)
                matz1_half(1, nc.scalar)

                # stage + load the first two W2 slabs before the rest of
                # the staging so L2 can start as soon as L1 drains
                slabs = {}
                matz2([0, 1], nc.scalar)
                for g in range(2):
                    slabs[g] = [slabp.tile([P, 512], BF, tag=f"w2slab{t}",
                                           name=f"w2s_{g}_{t}")
                                for t in range(32)]
                    for t in range(32):
                        nc.scalar.dma_start(
                            out=slabs[g][t][:],
                            in_=w2_jg[g][t * P:(t + 1) * P, :])
                matz2([2, 3, 4, 5, 6, 7], nc.scalar)

                # ---- Layer 0
                for b in range(BT):
                    for j in range(4):
                        ps = psp.tile([P, NB], F32, tag="ps",
                                      name=f"ps0_{b}_{j}")
                        for kt in range(8):
                            nc.tensor.matmul(
                                out=ps[:],
                                lhsT=w0sb[kt][:, j * P:(j + 1) * P],
                                rhs=xsb[kt][:, b * NB:(b + 1) * NB],
                                start=(kt == 0), stop=(kt == 7))
                        nc.vector.tensor_scalar_max(
                            out=h1sb[j][:, b * NB:(b + 1) * NB],
                            in0=ps[:], scalar1=0.0)
                    if b % 2 == 1:
                        pr = b // 2
                        for j in range(4):
                            nc.sync.dma_start(
                                out=h1c[pr][j * P:(j + 1) * P, :],
                                in_=h1sb[j][:, pr * 2 * NB:(pr + 1) * 2 * NB])
                        nc.gpsimd.collective_compute(
                            "AllGather", mybir.AluOpType.bypass,
                            replica_groups=RG,
                            ins=[h1c[pr].opt()], outs=[h1f[pr].opt()])

            # ---- Layer 1 (l0 pools closed)
            w1sb = [l1wp.tile([P, 512], BF, name=f"w1sb{t}")
                    for t in range(32)]
            for t in range(32):
                h, lk = (0, t) if t < 16 else (1, t - 16)
                nc.sync.dma_start(out=w1sb[t][:],
                                  in_=w1_h[h][lk * P:(lk + 1) * P, :])
            for pr in range(2):
                pss = [psp.tile([P, NB], F32, tag="ps",
                                name=f"ps1_{pr}_{g}") for g in range(8)]
                for kt in range(32):
                    rhs = l1rp.tile([P, 2 * NB], BF, tag="l1rhs",
                                    name=f"l1r_{pr}_{kt}")
                    nc.sync.dma_start(out=rhs[:],
                                      in_=h1f[pr][kt * P:(kt + 1) * P, :])
                    for b2 in range(2):
                        for j in range(4):
                            nc.tensor.matmul(
                                out=pss[b2 * 4 + j][:],
                                lhsT=w1sb[kt][:, j * P:(j + 1) * P],
                                rhs=rhs[:, b2 * NB:(b2 + 1) * NB],
                                start=(kt == 0), stop=(kt == 31))
                for b2 in range(2):
                    b = pr * 2 + b2
                    for j in range(4):
                        h2o = l1op.tile([P, NB], BF, tag="h2o",
                                        name=f"h2o_{b}_{j}")
                        nc.vector.tensor_scalar_max(
                            out=h2o[:], in0=pss[b2 * 4 + j][:], scalar1=0.0)
                        nc.sync.dma_start(out=h2c[b][j * P:(j + 1) * P, :],
                                          in_=h2o[:])
                    nc.gpsimd.collective_compute(
                        "AllGather", mybir.AluOpType.bypass,
                        replica_groups=RG,
                        ins=[h2c[b].opt()], outs=[h2f[b].opt()])

        # ---- Layer 2 (W2 slabbed by 512-wide j-group, h2 SBUF-resident)
        with tc.tile_pool(name="h2res", bufs=1) as h2p, \
             tc.tile_pool(name="l2o", bufs=8) as l2op:
            h2res = [h2p.tile([P, BATCH], BF, name=f"h2r{kt}")
                     for kt in range(32)]
            for b in range(BT):
                for kt in range(32):
                    nc.sync.dma_start(
                        out=h2res[kt][:, b * NB:(b + 1) * NB],
                        in_=h2f[b][kt * P:(kt + 1) * P, :])
            for jg in range(8):
                slab = slabs[jg]
                for b in range(BT):
                    pss = [psp.tile([P, NB], F32, tag="ps",
                                    name=f"ps2_{jg}_{b}_{j}")
                           for j in range(4)]
                    for kt in range(32):
                        for j in range(4):
                            nc.tensor.matmul(
                                out=pss[j][:],
                                lhsT=slab[kt][:, j * P:(j + 1) * P],
                                rhs=h2res[kt][:, b * NB:(b + 1) * NB],
                                start=(kt == 0), stop=(kt == 31))
                    for j in range(4):
                        osb = l2op.tile([P, NB], BF, tag="l2out",
                                        name=f"l2o_{jg}_{b}_{j}")
                        nc.vector.tensor_copy(out=osb[:], in_=pss[j][:])
                        nc.sync.dma_start(
                            out=out_d[(jg * 4 + j) * P:(jg * 4 + j + 1) * P,
                                      b * NB:(b + 1) * NB],
                            in_=osb[:])
                # load slab jg+2 now: its slot (slab jg's) was just freed
                if jg + 2 < 8:
                    g = jg + 2
                    slabs[g] = [slabp.tile([P, 512], BF, tag=f"w2slab{t}",
                                           name=f"w2s_{g}_{t}")
                                for t in range(32)]
                    for t in range(32):
                        nc.sync.dma_start(
                            out=slabs[g][t][:],
                            in_=w2_jg[g][t * P:(t + 1) * P, :])

    nc.compile()
    return nc


_NC_CACHE = None


def _get_nc():
    global _NC_CACHE
    if _NC_CACHE is None:
        _NC_CACHE = build_nc()
    return _NC_CACHE


def _prep_inputs(x, hw0, hw1, hw2):
    """Host prep: transpose x, build per-core periodic permuted-table slices."""
    x = np.asarray(x, np.float32)
    hws = [np.asarray(hw0, np.float32), np.asarray(hw1, np.float32),
           np.asarray(hw2, np.float32)]
    xT = np.ascontiguousarray(x.T).astype(ml_dtypes.bfloat16)

    per_core_hb = [[None] * 3 for _ in range(N_CORES)]
    for l in range(3):
        pl = PLANS[l]
        N, b = pl["N"], pl["b"]
        m_ext = pl["m_ext"]
        jw = JW[l]
        t0 = pl["u0"] - pl["shift"]          # core-0 slice start (in t-space)
        span = m_ext + (N_CORES - 1) * jw
        t = t0 + np.arange(span, dtype=np.int64)
        shared = hws[l][(b * t) % N].astype(ml_dtypes.bfloat16)
        for c in range(N_CORES):
            per_core_hb[c][l] = shared[c * jw: c * jw + m_ext]
    in_maps = []
    for c in range(N_CORES):
        in_maps.append({
            "xT": xT,
            "hb0": per_core_hb[c][0],
            "hb1": per_core_hb[c][1],
            "hb2": per_core_hb[c][2],
        })
    return in_maps


def kernel(x, hw0, hw1, hw2, trace=False):
    nc = _get_nc()
    in_maps = _prep_inputs(x, hw0, hw1, hw2)
    res = run_bass_kernel_spmd(nc, in_maps, list(range(N_CORES)), trace=trace)
    outs = [res.results[c]["outT"][:JW[2], :] for c in range(N_CORES)]
    full = np.concatenate(outs, axis=0)         # [32000, 2048] bf16
    out = np.ascontiguousarray(full.T).astype(np.float32)
    kernel.last_results = res
    return out
